# revision 1
# baseline (speedup 1.0000x reference)
"""CropPool2D Trainium2 kernel.

out[b, c] = mean of img_feats[b, c, y1:y2, x1:x2] for bbox (x1, y1, x2, y2).

Strategy (data-parallel over batch, 8 NeuronCores, 8 samples each):
  - Host derives, per sample: a fixed-size crop window (Hw x Ww = max crop
    extents over the whole batch), its band start row offset, column offset,
    and a window mask with 1/area folded in (0 outside the crop).
  - Device, per (sample, channel-group-of-128):
      * one contiguous-band DMA: img[s, 128ch, ys:ys+Hw, :] (rows are
        contiguous in memory -> 56*4B-granule descriptors at line rate),
        with the row offset taken from a register (same SPMD program on
        every core; offsets are data).
      * one fused DVE tensor_tensor_reduce: (window * mask) summed over the
        free dim -> the [128,1] per-channel crop mean (mask carries 1/area).
  - One final DMA scatters the [128, samples*groups] result tile to the
    [samples, C] output.
"""

import numpy as np

B, C, H, W = 64, 512, 56, 56
N_CORES = 8
BL = B // N_CORES  # samples per core
P = 128
G = C // P  # channel groups per sample

_prog_cache: dict = {}

# Offset sentinel far past the img tensor end: the DMA bounds check sees the
# whole AP out of range and (with bounds_check="skip_entire_dma") skips the
# transfer while still incrementing its semaphore.
_SENTINEL = 1 << 24


def _chunking(Hw: int):
    """Split the Hw-row band into n_chunks of ch_rows rows for skip-DMA."""
    n_chunks = min(4, Hw)
    ch_rows = -(-Hw // n_chunks)
    n_chunks = -(-Hw // ch_rows)
    return n_chunks, ch_rows


def _build_program(Hw: int, Ww: int, unroll: int = 1):
    """Build + compile the SPMD Bass program for window size Hw x Ww.

    unroll > 1 repeats the whole body (idempotent) for benchmarking: the
    marginal wall time per extra repetition is the kernel's steady-state
    device time without dispatch overhead.
    """
    import concourse.bacc as bacc
    import concourse.mybir as mybir
    import concourse.tile as tile
    from concourse.bass import ds

    f32 = mybir.dt.float32
    i32 = mybir.dt.int32

    nc = bacc.Bacc("TRN2", target_bir_lowering=False, debug=False)

    n_chunks, ch_rows = _chunking(Hw)

    img = nc.dram_tensor("img", [BL, C, H, W], f32, kind="ExternalInput").ap()
    # meta: per (sample, chunk): window-start offset, then a 0/1 load flag.
    meta = nc.dram_tensor(
        "meta", [1, 2 * BL * n_chunks], i32, kind="ExternalInput"
    ).ap()
    maskd = nc.dram_tensor("mask", [BL, Hw * Ww], f32, kind="ExternalInput").ap()
    outd = nc.dram_tensor("out", [BL, C], f32, kind="ExternalOutput").ap()

    with tile.TileContext(nc) as tc:
        with (
            tc.tile_pool(name="const", bufs=1) as constp,
            tc.tile_pool(name="bandp", bufs=4) as bandp,
            tc.tile_pool(name="maskp", bufs=2) as maskp,
            tc.tile_pool(name="prodp", bufs=2) as prodp,
            tc.tile_pool(name="outp", bufs=1) as outp,
        ):
            meta_sb = constp.tile([1, 2 * BL * n_chunks], i32)
            nc.sync.dma_start(meta_sb, meta)

            out_sb = outp.tile([P, BL * G], f32)

            # Pre-zero the band slots: chunk DMAs skipped at runtime leave
            # slot contents stale; zeroing once guarantees the masked-out
            # region is finite (0 * 0 = 0) even on first use.
            for _ in range(4):
                t = bandp.tile([P, Hw, W], f32, tag="band")
                nc.any.memset(t, 0.0)

            n_off = BL * n_chunks
            img_flat = img.rearrange("b c h w -> b c (h w)")
            # Chunk DMAs alternate between the two HWDGE rings (SP, ACT);
            # each ring's offset/flag registers live on its own engine.
            ring_eng = [
                (nc.sync, (mybir.EngineType.SP,)),
                (nc.scalar, (mybir.EngineType.Activation,)),
            ]

            for _rep in range(unroll):
                offs = []
                flags = []
                for s in range(BL):
                    for k in range(n_chunks):
                        i = s * n_chunks + k
                        eng = ring_eng[k % 2][1]
                        offs.append(
                            nc.values_load(
                                meta_sb[0:1, i : i + 1],
                                engines=eng,
                                min_val=0,
                                max_val=(H - Hw) * W + (W - Ww) + k * ch_rows * W,
                                skip_runtime_bounds_check=True,
                            )
                        )
                        flags.append(
                            nc.values_load(
                                meta_sb[0:1, n_off + i : n_off + i + 1],
                                engines=eng,
                                min_val=0,
                                max_val=1,
                                skip_runtime_bounds_check=True,
                            )
                        )
                for s in range(BL):
                    mask_sb = maskp.tile([P, Hw * Ww], f32)
                    nc.sync.dma_start(
                        mask_sb, maskd[s : s + 1, :].to_broadcast((P, Hw * Ww))
                    )
                    mask_v = mask_sb[:].rearrange("p (r x) -> p r x", x=Ww)
                    for g in range(G):
                        band = bandp.tile([P, Hw, W], f32, tag="band")
                        chan = img_flat[s, g * P : (g + 1) * P, :]
                        for k in range(n_chunks):
                            r0 = k * ch_rows
                            r1 = min(r0 + ch_rows, Hw)
                            # Last chunk stops at the window's last element so
                            # the span never crosses the channel end.
                            span = (
                                (r1 - r0) * W if r1 < Hw else (r1 - r0 - 1) * W + Ww
                            )
                            dst = band[:, r0:r1, :].rearrange("p r x -> p (r x)")[
                                :, 0:span
                            ]
                            i = s * n_chunks + k
                            ring_eng[k % 2][0].dma_start(
                                dst,
                                chan[:, ds(offs[i], span)],
                                cond=flags[i],
                            )

                        prod = prodp.tile([P, Hw * Ww], f32)
                        prod_v = prod[:].rearrange("p (r x) -> p r x", x=Ww)

                        col = s * G + g
                        # out = (window * 1.0) * mask; accum_out = sum(out).
                        # Window is the static strided view: rows stride W, cols 1.
                        nc.vector.scalar_tensor_tensor(
                            out=prod_v,
                            in0=band[:, :, 0:Ww],
                            scalar=1.0,
                            in1=mask_v,
                            op0=mybir.AluOpType.mult,
                            op1=mybir.AluOpType.mult,
                            accum_out=out_sb[:, col : col + 1],
                        )

            nc.sync.dma_start(
                outd.rearrange("s (g p) -> p s g", p=P),
                out_sb[:].rearrange("p (s g) -> p s g", g=G),
            )

    nc.compile()
    return nc


def _host_prep(img_feats: np.ndarray, bboxes: np.ndarray):
    bb = np.asarray(bboxes).astype(np.int64)
    x1, y1, x2, y2 = bb[:, 0], bb[:, 1], bb[:, 2], bb[:, 3]
    ch = y2 - y1
    cw = x2 - x1
    assert (ch > 0).all() and (cw > 0).all(), "invalid bboxes"
    Hw = int(ch.max())
    Ww = int(cw.max())
    ys = np.minimum(y1, H - Hw)
    xs = np.minimum(x1, W - Ww)
    dy = y1 - ys
    dx = x1 - xs
    inv_area = (1.0 / (ch * cw)).astype(np.float64)

    r = np.arange(Hw)[None, :, None]
    c = np.arange(Ww)[None, None, :]
    valid = (
        (r >= dy[:, None, None])
        & (r < (dy + ch)[:, None, None])
        & (c >= dx[:, None, None])
        & (c < (dx + cw)[:, None, None])
    )
    masks = (valid * inv_area[:, None, None]).astype(np.float32).reshape(B, Hw * Ww)

    # Per-chunk window-start offsets plus 0/1 flags; chunks with no valid
    # rows get flag 0 so the device DMA is predicated off (cond=).
    n_chunks, ch_rows = _chunking(Hw)
    base = ys * W + xs  # [B]
    offsets = np.empty((B, n_chunks), np.int64)
    flags = np.empty((B, n_chunks), np.int64)
    for k in range(n_chunks):
        r0, r1 = k * ch_rows, min((k + 1) * ch_rows, Hw)
        needed = (r0 < dy + ch) & (r1 > dy)  # chunk overlaps valid rows
        offsets[:, k] = base + r0 * W
        flags[:, k] = needed
    # Per-core rows [N_CORES, 2*BL*n_chunks]: that core's sample offsets
    # flattened, then its flags.
    meta = np.concatenate(
        [
            offsets.reshape(N_CORES, BL * n_chunks),
            flags.reshape(N_CORES, BL * n_chunks),
        ],
        axis=1,
    ).astype(np.int32)
    return Hw, Ww, masks, meta


def _run(img_feats: np.ndarray, bboxes: np.ndarray, **spmd_kwargs):
    from concourse.bass_utils import run_bass_kernel_spmd

    img = np.ascontiguousarray(np.asarray(img_feats), dtype=np.float32)
    assert img.shape == (B, C, H, W), img.shape
    Hw, Ww, masks, meta = _host_prep(img, bboxes)

    key = (Hw, Ww)
    if key not in _prog_cache:
        _prog_cache[key] = _build_program(Hw, Ww)
    nc = _prog_cache[key]

    in_maps = []
    for i in range(N_CORES):
        sl = slice(i * BL, (i + 1) * BL)
        in_maps.append(
            {
                "img": img[sl],
                "meta": meta[i : i + 1],
                "mask": masks[sl],
            }
        )

    res = run_bass_kernel_spmd(
        nc, in_maps, core_ids=list(range(N_CORES)), **spmd_kwargs
    )
    out = np.concatenate([res.results[i]["out"] for i in range(N_CORES)], axis=0)
    return out.astype(np.float32), res


def kernel(img_feats: np.ndarray, bboxes: np.ndarray) -> np.ndarray:
    out, _ = _run(img_feats, bboxes)
    return out



# revision 3
# speedup vs baseline: 1.6675x; 1.6675x over previous
"""CropPool2D Trainium2 kernel.

out[b, c] = mean of img_feats[b, c, y1:y2, x1:x2] for bbox (x1, y1, x2, y2).

Strategy (data-parallel over batch, 8 NeuronCores, 8 samples each):
  - Samples are regrouped into 8 "slots" of 8 (one sample per core per
    slot), clustered by crop height so each slot's static window shape
    (slot-max ch x slot-max cw) is tight. Every core runs the same SPMD
    program; the sample->-(core, slot) permutation is undone on host.
  - Device, per (slot, sample): ONE DMA loads the bf16 crop window for
    all 512 channels: [128 part, 4 grp, ch, cw] with dynamic y/x window
    offsets taken from registers. Crop rows are the only contiguous
    runs, so the DMA cost is descriptor-bound (512*ch rows); bf16 makes
    every row descriptor hit the minimum-descriptor-time floor.
  - One packed broadcast DMA ships all 8 window masks (inv_area inside
    the crop, 0 on the slot-window overage; windows are clamped inside
    the image so overage is always finite image data - no pre-zeroing).
  - Per (slot, group-of-128-channels): one fused DVE
    scalar_tensor_tensor (window * mask) with accum_out -> the [128,1]
    per-channel crop mean. One final DMA scatters [128, 32] -> [8, 512].
"""

import numpy as np
import ml_dtypes

B, C, H, W = 64, 512, 56, 56
N_CORES = 8
BL = B // N_CORES  # samples per core == slots
P = 128
G = C // P  # channel groups per sample

_prog_cache: dict = {}


def _assign_slots(ch: np.ndarray, cw: np.ndarray):
    """Group the 64 samples into 8 slots of 8, one member per core.

    DMA cost is descriptor-bound: per slot ~ 512*max_ch descriptors at
    the floor rate, so sorting by crop height and taking octiles
    minimizes sum(max_ch). Within each octile, sorting by width tightens
    max_cw for the (secondary) DVE/mask cost.
    """
    order = np.lexsort((cw, ch))[::-1]  # ch desc, then cw desc
    groups = [order[j * N_CORES : (j + 1) * N_CORES] for j in range(BL)]
    shapes = [(int(ch[g].max()), int(cw[g].max())) for g in groups]
    return groups, shapes


def _build_program(shapes, unroll: int = 1):
    """Build + compile the SPMD Bass program for the 8 slot shapes."""
    import concourse.bacc as bacc
    import concourse.mybir as mybir
    import concourse.tile as tile
    from concourse.bass import ds

    f32 = mybir.dt.float32
    bf16 = mybir.dt.bfloat16
    i32 = mybir.dt.int32

    nc = bacc.Bacc("TRN2", target_bir_lowering=False, debug=False)

    areas = [h * w for h, w in shapes]
    tot_area = sum(areas)
    moffs = np.cumsum([0] + areas).tolist()

    img = nc.dram_tensor("img", [BL, C, H, W], bf16, kind="ExternalInput").ap()
    # meta: per slot: y window offset, then x window offset.
    meta = nc.dram_tensor("meta", [1, 2 * BL], i32, kind="ExternalInput").ap()
    maskd = nc.dram_tensor("mask", [1, tot_area], bf16, kind="ExternalInput").ap()
    outd = nc.dram_tensor("out", [BL, C], f32, kind="ExternalOutput").ap()

    with tile.TileContext(nc) as tc:
        with (
            tc.tile_pool(name="const", bufs=1) as constp,
            tc.tile_pool(name="bandp", bufs=1) as bandp,
            tc.tile_pool(name="prodp", bufs=2) as prodp,
            tc.tile_pool(name="outp", bufs=1) as outp,
        ):
            meta_sb = constp.tile([1, 2 * BL], i32)
            nc.sync.dma_start(meta_sb, meta)

            mask_sb = constp.tile([P, tot_area], bf16)
            nc.gpsimd.dma_start(
                mask_sb, maskd[0:1, :].to_broadcast((P, tot_area))
            )

            out_sb = outp.tile([P, BL * G], f32)

            # Band DMAs alternate between the SP and ACT HWDGE rings; the
            # window-offset registers live on the issuing engine.
            ring_eng = [
                (nc.sync, (mybir.EngineType.SP,)),
                (nc.scalar, (mybir.EngineType.Activation,)),
            ]
            bands = [
                bandp.tile([P, G * h * w], bf16, name=f"band{j}", tag=f"band{j}")
                for j, (h, w) in enumerate(shapes)
            ]

            for _rep in range(unroll):
                for j, (h, w) in enumerate(shapes):
                    eng, etypes = ring_eng[j % 2]
                    yo = nc.values_load(
                        meta_sb[0:1, j : j + 1],
                        engines=etypes,
                        min_val=0,
                        max_val=H - h,
                        skip_runtime_bounds_check=True,
                    )
                    xo = nc.values_load(
                        meta_sb[0:1, BL + j : BL + j + 1],
                        engines=etypes,
                        min_val=0,
                        max_val=W - w,
                        skip_runtime_bounds_check=True,
                    )
                    # src AP [c, h, w] pairs with dst [p, g, h, w]: c = 4p+g.
                    eng.dma_start(
                        bands[j][:].rearrange("p (g h w) -> p g h w", g=G, h=h),
                        img[j, :, ds(yo, h), ds(xo, w)],
                    )

                for j, (h, w) in enumerate(shapes):
                    band_v = bands[j][:].rearrange(
                        "p (g h w) -> p g h w", g=G, h=h
                    )
                    mask_v = mask_sb[:, moffs[j] : moffs[j + 1]].rearrange(
                        "p (h w) -> p h w", w=w
                    )
                    for g in range(G):
                        prod = prodp.tile([P, h * w], bf16)
                        col = j * G + g
                        # out = (window * 1.0) * mask; accum = sum(out).
                        nc.vector.scalar_tensor_tensor(
                            out=prod[:].rearrange("p (h w) -> p h w", w=w),
                            in0=band_v[:, g],
                            scalar=1.0,
                            in1=mask_v,
                            op0=mybir.AluOpType.mult,
                            op1=mybir.AluOpType.mult,
                            accum_out=out_sb[:, col : col + 1],
                        )

            # out_sb[p, j*G+g] holds channel c = 4p+g of slot-sample j.
            nc.sync.dma_start(
                outd.rearrange("s (p g) -> p s g", g=G),
                out_sb[:].rearrange("p (s g) -> p s g", g=G),
            )

    nc.compile()
    return nc


def _host_prep(bboxes: np.ndarray):
    bb = np.asarray(bboxes).astype(np.int64)
    x1, y1, x2, y2 = bb[:, 0], bb[:, 1], bb[:, 2], bb[:, 3]
    ch = y2 - y1
    cw = x2 - x1
    assert (ch > 0).all() and (cw > 0).all(), "invalid bboxes"
    groups, shapes = _assign_slots(ch, cw)

    inv_area = 1.0 / (ch * cw).astype(np.float64)

    areas = [h * w for h, w in shapes]
    tot_area = sum(areas)
    meta = np.zeros((N_CORES, 2 * BL), np.int32)
    masks = np.zeros((N_CORES, tot_area), np.float32)
    off = 0
    for j, (hj, wj) in enumerate(shapes):
        g = groups[j]  # 8 sample ids, one per core
        ys = np.minimum(y1[g], H - hj)  # window start (clamped in-image)
        xs = np.minimum(x1[g], W - wj)
        meta[:, j] = ys
        meta[:, BL + j] = xs
        dy = (y1[g] - ys)[:, None, None]
        dx = (x1[g] - xs)[:, None, None]
        r = np.arange(hj)[None, :, None]
        c = np.arange(wj)[None, None, :]
        valid = (
            (r >= dy)
            & (r < dy + ch[g][:, None, None])
            & (c >= dx)
            & (c < dx + cw[g][:, None, None])
        )
        m = valid * inv_area[g][:, None, None]
        masks[:, off : off + hj * wj] = m.reshape(N_CORES, hj * wj)
        off += hj * wj

    return groups, shapes, meta, masks.astype(ml_dtypes.bfloat16)


def _run(img_feats: np.ndarray, bboxes: np.ndarray, **spmd_kwargs):
    from concourse.bass_utils import run_bass_kernel_spmd

    img = np.asarray(img_feats)
    assert img.shape == (B, C, H, W), img.shape
    img16 = np.ascontiguousarray(img.astype(ml_dtypes.bfloat16))
    groups, shapes, meta, masks = _host_prep(bboxes)

    key = tuple(shapes)
    if key not in _prog_cache:
        _prog_cache[key] = _build_program(list(shapes))
    nc = _prog_cache[key]

    in_maps = []
    for i in range(N_CORES):
        sample_ids = [groups[j][i] for j in range(BL)]
        in_maps.append(
            {
                "img": img16[sample_ids],
                "meta": meta[i : i + 1],
                "mask": masks[i : i + 1],
            }
        )

    res = run_bass_kernel_spmd(
        nc, in_maps, core_ids=list(range(N_CORES)), **spmd_kwargs
    )
    out = np.empty((B, C), np.float32)
    for i in range(N_CORES):
        core_out = res.results[i]["out"]  # [BL, C] in slot order
        for j in range(BL):
            out[groups[j][i]] = core_out[j]
    return out, res


def kernel(img_feats: np.ndarray, bboxes: np.ndarray) -> np.ndarray:
    out, _ = _run(img_feats, bboxes)
    return out


# revision 4
# speedup vs baseline: 2.4574x; 1.4737x over previous
"""CropPool2D Trainium2 kernel.

out[b, c] = mean of img_feats[b, c, y1:y2, x1:x2] for bbox (x1, y1, x2, y2).

Strategy (data-parallel over batch, 8 NeuronCores, 8 samples each):
  - Samples are regrouped into 8 "slots" of 8 (one sample per core per
    slot), clustered by crop height so each slot's static window shape
    (slot-max ch x slot-max cw) is tight. Every core runs the same SPMD
    program; the sample->(core, slot) permutation is undone on host.
  - DMA cost on TRN2 is dominated by a ~17ns fixed cost per descriptor
    (contiguous run), so per (slot, sample) ONE DMA loads a contiguous
    bf16 span per channel: img[s, c, yo*W+xo : ... + (h-1)*W + w]. The
    crop window is then the strided view [h rows x first w cols] of
    that span (wrapped columns are loaded but never read).
  - One packed broadcast DMA ships all 8 window masks (inv_area inside
    the crop, 0 on the slot-window overage; windows are clamped inside
    the image so overage is always finite image data - no pre-zeroing).
  - Per (slot, group-of-128-channels): one fused DVE
    scalar_tensor_tensor (window * mask) with accum_out -> the [128,1]
    per-channel crop mean. One final DMA scatters [128, 32] -> [8, 512].
"""

import numpy as np
import ml_dtypes

B, C, H, W = 64, 512, 56, 56
N_CORES = 8
BL = B // N_CORES  # samples per core == slots
P = 128
G = C // P  # channel groups per sample

_prog_cache: dict = {}

# Band DMAs alternate between the SP and ACT HWDGE rings; ring r owns
# slots r, r+2, r+4, r+6 and loads its 4 window offsets in one go.
_RING_SLOTS = [[0, 2, 4, 6], [1, 3, 5, 7]]


def _assign_slots(ch: np.ndarray, cw: np.ndarray):
    """Group the 64 samples into 8 slots of 8, one member per core.

    DMA cost is span-bound (~ slot-max ch), so sorting by crop height
    and taking octiles minimizes sum(max_ch). The secondary cw sort
    tightens max_cw for the DVE/mask cost.
    """
    order = np.lexsort((cw, ch))[::-1]  # ch desc, then cw desc
    groups = [order[j * N_CORES : (j + 1) * N_CORES] for j in range(BL)]
    shapes = [(int(ch[g].max()), int(cw[g].max())) for g in groups]
    return groups, shapes


def _build_program(shapes, unroll: int = 1):
    """Build + compile the SPMD Bass program for the 8 slot shapes."""
    import concourse.bacc as bacc
    import concourse.mybir as mybir
    import concourse.tile as tile
    from concourse.bass import ds

    f32 = mybir.dt.float32
    bf16 = mybir.dt.bfloat16
    i32 = mybir.dt.int32

    nc = bacc.Bacc("TRN2", target_bir_lowering=False, debug=False)

    areas = [h * w for h, w in shapes]
    spans = [(h - 1) * W + w for h, w in shapes]
    tot_area = sum(areas)
    moffs = np.cumsum([0] + areas).tolist()

    img = nc.dram_tensor("img", [BL, C, H, W], bf16, kind="ExternalInput").ap()
    # meta: flat window-start offsets (yo*W+xo), SP-ring slots then ACT's.
    meta = nc.dram_tensor("meta", [1, BL], i32, kind="ExternalInput").ap()
    maskd = nc.dram_tensor("mask", [1, tot_area], bf16, kind="ExternalInput").ap()
    outd = nc.dram_tensor("out", [BL, C], f32, kind="ExternalOutput").ap()

    img_flat = img.rearrange("b c h w -> b c (h w)")

    with tile.TileContext(nc) as tc:
        with (
            tc.tile_pool(name="const", bufs=1) as constp,
            tc.tile_pool(name="bandp", bufs=1) as bandp,
            tc.tile_pool(name="prodp", bufs=2) as prodp,
            tc.tile_pool(name="outp", bufs=1) as outp,
        ):
            meta_sb = constp.tile([1, BL], i32)
            nc.sync.dma_start(meta_sb, meta)

            mask_sb = constp.tile([P, tot_area], bf16)
            nc.gpsimd.dma_start(mask_sb, maskd[0:1, :].to_broadcast((P, tot_area)))

            out_sb = outp.tile([P, BL * G], f32)

            rings = [
                (nc.sync, (mybir.EngineType.SP,)),
                (nc.scalar, (mybir.EngineType.Activation,)),
            ]
            # Band tiles hold full-width rows; the DMA writes the leading
            # span only, and compute reads only the first w columns of
            # each row, so the unwritten tail is never consumed.
            bands = [
                bandp.tile([P, G, h, W], bf16, name=f"band{j}", tag=f"band{j}")
                for j, (h, w) in enumerate(shapes)
            ]

            for _rep in range(unroll):
                offs = [None] * BL
                for r, (eng, etypes) in enumerate(rings):
                    _, vals = nc.values_load_multi_w_load_instructions(
                        meta_sb[0:1, 4 * r : 4 * (r + 1)],
                        engines=etypes,
                        min_val=0,
                        max_val=H * W - 1,
                        skip_runtime_bounds_check=True,
                    )
                    for k, j in enumerate(_RING_SLOTS[r]):
                        # Tighten the per-slot bound for the AP check.
                        offs[j] = nc.s_assert_within(
                            vals[k],
                            min_val=0,
                            max_val=H * W - spans[j],
                            skip_runtime_assert=True,
                        )

                for j, (h, w) in enumerate(shapes):
                    eng, _ = rings[j % 2]
                    dst = bands[j][:].rearrange("p g h x -> p (g h x)")
                    dst = dst.rearrange("p (g s) -> p g s", g=G)[:, :, 0 : spans[j]]
                    eng.dma_start(dst, img_flat[j, :, ds(offs[j], spans[j])])

                for j, (h, w) in enumerate(shapes):
                    mask_v = mask_sb[:, moffs[j] : moffs[j + 1]].rearrange(
                        "p (h w) -> p h w", w=w
                    )
                    for g in range(G):
                        prod = prodp.tile([P, h * w], bf16, name=f"prod{j}_{g}")
                        col = j * G + g
                        # out = (window * 1.0) * mask; accum = sum(out).
                        nc.vector.scalar_tensor_tensor(
                            out=prod[:].rearrange("p (h w) -> p h w", w=w),
                            in0=bands[j][:, g, :, 0:w],
                            scalar=1.0,
                            in1=mask_v,
                            op0=mybir.AluOpType.mult,
                            op1=mybir.AluOpType.mult,
                            accum_out=out_sb[:, col : col + 1],
                        )

            # out_sb[p, j*G+g] holds channel c = 4p+g of slot-sample j.
            nc.sync.dma_start(
                outd.rearrange("s (p g) -> p s g", g=G),
                out_sb[:].rearrange("p (s g) -> p s g", g=G),
            )

    nc.compile()
    return nc


def _host_prep(bboxes: np.ndarray):
    bb = np.asarray(bboxes).astype(np.int64)
    x1, y1, x2, y2 = bb[:, 0], bb[:, 1], bb[:, 2], bb[:, 3]
    ch = y2 - y1
    cw = x2 - x1
    assert (ch > 0).all() and (cw > 0).all(), "invalid bboxes"
    groups, shapes = _assign_slots(ch, cw)

    inv_area = 1.0 / (ch * cw).astype(np.float64)

    areas = [h * w for h, w in shapes]
    tot_area = sum(areas)
    meta = np.zeros((N_CORES, BL), np.int32)
    masks = np.zeros((N_CORES, tot_area), np.float32)
    meta_pos = {j: r * 4 + k for r, sl in enumerate(_RING_SLOTS) for k, j in enumerate(sl)}
    off = 0
    for j, (hj, wj) in enumerate(shapes):
        g = groups[j]  # 8 sample ids, one per core
        ys = np.minimum(y1[g], H - hj)  # window start (clamped in-image)
        xs = np.minimum(x1[g], W - wj)
        meta[:, meta_pos[j]] = ys * W + xs
        dy = (y1[g] - ys)[:, None, None]
        dx = (x1[g] - xs)[:, None, None]
        r = np.arange(hj)[None, :, None]
        c = np.arange(wj)[None, None, :]
        valid = (
            (r >= dy)
            & (r < dy + ch[g][:, None, None])
            & (c >= dx)
            & (c < dx + cw[g][:, None, None])
        )
        m = valid * inv_area[g][:, None, None]
        masks[:, off : off + hj * wj] = m.reshape(N_CORES, hj * wj)
        off += hj * wj

    return groups, shapes, meta, masks.astype(ml_dtypes.bfloat16)


def _run(img_feats: np.ndarray, bboxes: np.ndarray, **spmd_kwargs):
    from concourse.bass_utils import run_bass_kernel_spmd

    img = np.asarray(img_feats)
    assert img.shape == (B, C, H, W), img.shape
    img16 = np.ascontiguousarray(img.astype(ml_dtypes.bfloat16))
    groups, shapes, meta, masks = _host_prep(bboxes)

    key = tuple(shapes)
    if key not in _prog_cache:
        _prog_cache[key] = _build_program(list(shapes))
    nc = _prog_cache[key]

    in_maps = []
    for i in range(N_CORES):
        sample_ids = [groups[j][i] for j in range(BL)]
        in_maps.append(
            {
                "img": img16[sample_ids],
                "meta": meta[i : i + 1],
                "mask": masks[i : i + 1],
            }
        )

    res = run_bass_kernel_spmd(
        nc, in_maps, core_ids=list(range(N_CORES)), **spmd_kwargs
    )
    out = np.empty((B, C), np.float32)
    for i in range(N_CORES):
        core_out = res.results[i]["out"]  # [BL, C] in slot order
        for j in range(BL):
            out[groups[j][i]] = core_out[j]
    return out, res


def kernel(img_feats: np.ndarray, bboxes: np.ndarray) -> np.ndarray:
    out, _ = _run(img_feats, bboxes)
    return out
